# revision 32
# baseline (speedup 1.0000x reference)
"""Causal self-attention (B=4, T=2048, C=1024, H=16, Dh=64) on 8 NeuronCores.

Sharding: core = g*4 + b  (b = batch 0..3, g = head-group 0..1).
Each core computes one batch's attention for 8 heads (a 512-wide slice of
qkv columns) plus the matching row-slice of the output projection; the two
head-group partial projections per batch are summed on the host.

v3: bf16 operands everywhere (fp32 PSUM), software-pipelined phases.
  - prologue: v for all heads + qT/kT for pair 0 only.
  - attention pair j runs interleaved with the qT/kT matmuls for pair
    j+1 (pairs 0-2) or the output-projection tiles whose attnT rows are
    complete (pair 3) — phase 2 is exp(ACT)-bound, so the spare PE
    cycles absorb the neighbouring phases nearly for free.
  - attention core: S^T = kT.T @ qT (two heads row-packed via
    tile_position, concurrent), exp on ACT -> bf16 pt, causal mask mult
    on DVE, outT accumulated with a ones column for the denominator,
    reciprocal via batched DVE (pairs 0-2) or ACT Ln/Exp (pair 3 tail),
    DRAM-bounce partition broadcast, DVE mult writes attnT directly.
"""

from contextlib import ExitStack

import numpy as np
import ml_dtypes

import concourse.bass as bass
import concourse.mybir as mybir
import concourse.tile as tile
from concourse.tile import add_dep_helper
from concourse.bass_utils import run_bass_kernel_spmd
from concourse.vector_clock import ScopedClock

FP = mybir.dt.float32
BF = mybir.dt.bfloat16
AF = mybir.ActivationFunctionType

T = 2048
C = 1024
HL = 8          # heads per core
DH = 64
G = 512         # q (=k=v) column width per core
N_CORES = 8


_NO_SPLIT_OPCODES = ()


def _split_multi_waits(nc, max_waits=1):
    """This walrus build rejects >1 sync wait per engine instruction.
    Hoist extra waits onto single-wait NOPs inserted just before the
    instruction on the same engine (engine streams execute in bb order)."""
    fixes = {}
    bbs = [bb for fn in nc.m.functions for bb in fn.blocks]
    for bb in bbs:
        for inst in list(bb.instructions):
            si = inst.sync_info
            if si is None or not si.on_wait:
                continue
            waits = list(si.on_wait)
            if len(waits) <= max_waits:
                continue
            op = str(inst.opcode)
            if any(t in op for t in _NO_SPLIT_OPCODES):
                continue
            if inst.engine not in nc.engines:
                continue
            extra, keep = waits[:-max_waits], waits[-max_waits:]
            nops = []
            for w in extra:
                nop = nc.engines[inst.engine].nop(nofuse=True).ins
                nop.sync_info = mybir.SyncInfo(on_wait=[w], on_update=[])
                nops.append(nop)
            si.on_wait = keep
            fixes[inst.name] = nops
    if not fixes:
        return
    appended = {n.name for nops in fixes.values() for n in nops}
    for bb in bbs:
        new_insts = []
        for inst in bb.instructions:
            if inst.name in appended:
                continue
            new_insts.extend(fixes.get(inst.name, ()))
            new_insts.append(inst)
        bb.instructions = new_insts


def _build_nc():
    nc = bass.Bass("TRN2", target_bir_lowering=False, debug=False)
    xT = nc.dram_tensor("xT", [C, T], BF, kind="ExternalInput")
    wqk = nc.dram_tensor("wqk", [C, 2 * G], BF, kind="ExternalInput")
    wv = nc.dram_tensor("wv", [C, G], BF, kind="ExternalInput")
    wproj = nc.dram_tensor("wproj", [G, C], BF, kind="ExternalInput")
    maskW = nc.dram_tensor("maskW", [128, 896], BF, kind="ExternalInput")
    out = nc.dram_tensor("out", [T, C], FP, kind="ExternalOutput")

    with tile.TileContext(nc) as tc:
        with (
            tc.tile_pool(name="persist", bufs=1) as persist,
            tc.tile_pool(name="const", bufs=1) as const,
            tc.tile_pool(name="wqk", bufs=16) as wqkp,
            tc.tile_pool(name="wv", bufs=8) as wvp,
            tc.tile_pool(name="xh", bufs=2) as xpool,
            tc.tile_pool(name="gps", bufs=2, space="PSUM") as gp_ps,
            tc.tile_pool(name="outsb", bufs=4) as outsb,
        ):
            maskW_sb = const.tile([128, 896], BF)
            qT_sb = persist.tile([128, 4, T], BF, tag="qT")
            kT_sb = persist.tile([128, 4, T], BF, tag="kT")
            v_sb = persist.tile([128, 16, HL, DH + 1], BF, tag="v")
            attnT_sb = persist.tile([128, 4, T], BF, tag="attnT")
            wproj_sb = persist.tile([128, 4, C], BF, tag="wp")

            # ---------------- input loads ----------------
            # first-needed data first, split per 128-row k slice so the
            # first matmul starts after ~256KB: wqk k-tiles on the ACT
            # queue, x k-slices on the SP queue.
            wqk_sb = {}
            xh_t = {}

            def load_xh(half):
                xt = xpool.tile([128, 8, 1024], BF, tag="x", name=f"x{half}")
                t0 = half * 1024
                for k in range(8):
                    nc.sync.dma_start(
                        out=xt[:, k, :],
                        in_=xT[k * 128 : (k + 1) * 128, t0 : t0 + 1024],
                    )
                xh_t[half] = xt

            def load_wqk(mh):
                for k in range(8):
                    wt = wqkp.tile([128, 512], BF, tag="wqk", name=f"wqk{k}_{mh}")
                    nc.scalar.dma_start(
                        out=wt[:],
                        in_=wqk[k * 128 : (k + 1) * 128, mh * 512 : (mh + 1) * 512],
                    )
                    wqk_sb[(k, mh)] = wt

            load_wqk(0)
            load_xh(0)
            load_wqk(1)
            load_xh(1)
            nc.scalar.dma_start(out=maskW_sb[:], in_=maskW[:])
            wv_sb = [wvp.tile([128, G], BF, tag="wv", name=f"wv{k}") for k in range(8)]
            for k in range(8):
                nc.sync.dma_start(
                    out=wv_sb[k][:], in_=wv[k * 128 : (k + 1) * 128, :]
                )
            for k in range(4):
                nc.scalar.dma_start(
                    out=wproj_sb[:, k], in_=wproj[k * 128 : (k + 1) * 128, :]
                )
            # ones columns for the softmax denominator
            nc.vector.tensor_copy(
                v_sb[:, :, :, DH],
                maskW_sb[:, 895:896].to_broadcast((128, 16 * HL)),
            )

            # one qT/kT unit: 8 PE matmuls + 1 DVE eviction for a 512-wide
            # t block of q (which=0) or k (which=1) columns of one pair.
            def qkT_unit(j, half, n, which):
                m = j + 4 * which
                ps = gp_ps.tile([128, 512], FP, tag="gp", name=f"qk{j}_{half}_{n}_{which}")
                for k in range(8):
                    nc.tensor.matmul(
                        ps[:],
                        wqk_sb[(k, m // 4)][:, (m % 4) * 128 : (m % 4 + 1) * 128],
                        xh_t[half][:, k, n * 512 : (n + 1) * 512],
                        start=(k == 0),
                        stop=(k == 7),
                    )
                dst = qT_sb if which == 0 else kT_sb
                tq = half * 1024 + n * 512
                nc.vector.tensor_copy(dst[:, j, tq : tq + 512], ps[:])

            def qkT_units(j):
                for half in range(2):
                    for n in range(2):
                        for which in (0, 1):
                            yield (j, half, n, which)

            def qkT_halves(j, half, n, which):
                # the same unit as two 4-matmul emissions sharing one PSUM
                # tile, so a full-mode filler matmul can sit between every
                # S-pair and its AVs (absorbs the PE mode-switch drain)
                m = j + 4 * which
                st = {}

                def h1():
                    st["ps"] = gp_ps.tile(
                        [128, 512], FP, tag="gp", name=f"qk{j}_{half}_{n}_{which}"
                    )
                    for k in range(4):
                        nc.tensor.matmul(
                            st["ps"][:],
                            wqk_sb[(k, m // 4)][:, (m % 4) * 128 : (m % 4 + 1) * 128],
                            xh_t[half][:, k, n * 512 : (n + 1) * 512],
                            start=(k == 0),
                            stop=False,
                        )

                def h2():
                    for k in range(4, 8):
                        nc.tensor.matmul(
                            st["ps"][:],
                            wqk_sb[(k, m // 4)][:, (m % 4) * 128 : (m % 4 + 1) * 128],
                            xh_t[half][:, k, n * 512 : (n + 1) * 512],
                            start=False,
                            stop=(k == 7),
                        )
                    dst = qT_sb if which == 0 else kT_sb
                    tq = half * 1024 + n * 512
                    nc.vector.tensor_copy(dst[:, j, tq : tq + 512], st["ps"][:])

                return [h1, h2]

            def proj_halves(tt):
                ot = outsb.tile([128, 1024], FP, tag="ot", name=f"ot{tt}")

                def mk(n):
                    def h():
                        ps = gp_ps.tile(
                            [128, 512], FP, tag="gp", name=f"pj{tt}_{n}"
                        )
                        for k in range(4):
                            nc.tensor.matmul(
                                ps[:],
                                attnT_sb[:, k, tt * 128 : (tt + 1) * 128],
                                wproj_sb[:, k, n * 512 : (n + 1) * 512],
                                start=(k == 0),
                                stop=(k == 3),
                            )
                        if n == 0:
                            nc.scalar.copy(ot[:, 0:512], ps[:])
                        else:
                            nc.vector.tensor_copy(ot[:, 512:1024], ps[:])
                        nc.sync.dma_start(
                            out=out[tt * 128 : (tt + 1) * 128, n * 512 : (n + 1) * 512],
                            in_=ot[:, n * 512 : (n + 1) * 512],
                        )

                    return h

                return [mk(0), mk(1)]

            # one proj unit: 2x(4 PE matmuls + eviction) + 1 output DMA
            # for one 128-row t tile. Valid once attnT pair 3 covers it.
            def proj_unit(tt, use_act=False):
                ot = outsb.tile([128, 1024], FP, tag="ot", name=f"ot{tt}")
                for n in range(2):
                    ps = gp_ps.tile([128, 512], FP, tag="gp", name=f"pj{tt}_{n}")
                    for k in range(4):
                        nc.tensor.matmul(
                            ps[:],
                            attnT_sb[:, k, tt * 128 : (tt + 1) * 128],
                            wproj_sb[:, k, n * 512 : (n + 1) * 512],
                            start=(k == 0),
                            stop=(k == 3),
                        )
                    if n == 0:
                        nc.scalar.copy(ot[:, 0:512], ps[:])
                    else:
                        nc.vector.tensor_copy(ot[:, 512:1024], ps[:])
                    nc.sync.dma_start(
                        out=out[tt * 128 : (tt + 1) * 128, n * 512 : (n + 1) * 512],
                        in_=ot[:, n * 512 : (n + 1) * 512],
                    )

            def v_unit(tt):
                half, mt = tt // 8, tt % 8
                ps = gp_ps.tile([128, 512], FP, tag="gp", name=f"v{half}_{mt}")
                for k in range(8):
                    nc.tensor.matmul(
                        ps[:],
                        xh_t[half][:, k, mt * 128 : (mt + 1) * 128],
                        wv_sb[k][:],
                        start=(k == 0),
                        stop=(k == 7),
                    )
                nc.vector.tensor_copy(v_sb[:, tt, :, 0:DH], ps[:])

            # ---------------- prologue: qT/kT pair 0 + v tiles 0-3 --------
            # (v tiles 4-15 are computed inside pair 0, each ahead of the
            # q-block that first consumes it — keeps the DMA-bound start
            # short and fills pair 0's exp-bound PE slack)
            for half in range(2):
                for which in (0, 1):
                    for n in range(2):
                        qkT_unit(0, half, n, which)
            for tt in range(4):
                v_unit(tt)

            # ---------------- attention + interleaved neighbours --------
            with (
                tc.tile_pool(name="sps", bufs=2, space="PSUM") as spool,
                tc.tile_pool(name="ope", bufs=1, space="PSUM") as opool_e,
                tc.tile_pool(name="opo", bufs=1, space="PSUM") as opool_o,
                tc.tile_pool(name="pt", bufs=6) as ptpool,
                tc.tile_pool(name="rec", bufs=2) as recpool,
                tc.tile_pool(name="stg", bufs=12) as stgpool,
                tc.tile_pool(name="bc", bufs=2) as bpool,
                tc.tile_pool(name="dscr", bufs=2, space="DRAM") as dpool,
            ):
                def norm_bcast_mul(j, stgs, recip, nrows):
                    # bounce recip rows through DRAM for the partition
                    # broadcast, then one DVE mult per (head, q-block)
                    # writes attnT (bf16) directly.
                    dscr = dpool.tile([8, 512], FP, tag="d")
                    nc.sync.dma_start(out=dscr[0:nrows, :], in_=recip[0:nrows, :])
                    for hh, qb, stg, ri in stgs:
                        bc = bpool.tile([64, 512], FP, tag="bc")
                        nc.sync.dma_start(
                            out=bc[:],
                            in_=dscr[ri : ri + 1, :].to_broadcast((64, 512)),
                        )
                        nc.vector.tensor_mul(
                            attnT_sb[
                                hh * 64 : (hh + 1) * 64,
                                j,
                                qb * 512 : qb * 512 + 512,
                            ],
                            stg[0:64, :],
                            bc[:],
                        )

                def norm_flush(j, stgs, collect):
                    # batched softmax denominators for a whole pair (DVE)
                    recip = recpool.tile([8, 512], FP, tag="rec")
                    nc.vector.reciprocal(recip[:], collect[:])
                    norm_bcast_mul(j, stgs, recip, 8)

                def norm_last_final(j, qb, oTs):
                    # very last q-block: the exp stream is done, so the ACT
                    # Ln/Exp reciprocal (straight off the PSUM ones-row) is
                    # the lowest-latency path to free the tail
                    dscr = dpool.tile([8, 512], FP, tag="d")
                    muls = []
                    for hh, oT in oTs:
                        recip = recpool.tile(
                            [1, 512], FP, tag="rec2", name=f"r3_{qb}_{hh}"
                        )
                        nc.scalar.activation(recip[:], oT[64:65, :], AF.Ln)
                        stg = stgpool.tile([65, 512], FP, tag="stg")
                        nc.vector.tensor_copy(stg[0:64, :], oT[0:64, :])
                        nc.scalar.activation(recip[:], recip[:], AF.Exp, scale=-1.0)
                        nc.sync.dma_start(out=dscr[hh : hh + 1, :], in_=recip[:])
                        muls.append((hh, stg))
                    for hh, stg in muls:
                        bc = bpool.tile([64, 512], FP, tag="bc")
                        nc.sync.dma_start(
                            out=bc[:],
                            in_=dscr[hh : hh + 1, :].to_broadcast((64, 512)),
                        )
                        nc.vector.tensor_mul(
                            attnT_sb[
                                hh * 64 : (hh + 1) * 64,
                                j,
                                qb * 512 : qb * 512 + 512,
                            ],
                            stg[0:64, :],
                            bc[:],
                        )

                pending = None
                for j in range(4):  # head pair: heads 2j (parts 0-63), 2j+1
                    last_pair = j == 3
                    stgs = []
                    collect = recpool.tile([8, 512], FP, tag="col")
                    # work interleaved into this pair's PE slack:
                    # qT/kT of the next pair, or proj tiles during pair 3
                    filler = (
                        [h for u in qkT_units(j + 1) for h in qkT_halves(*u)]
                        if j < 3
                        else []
                    )
                    fill_iv = 1  # emit one half-unit per tkb slot
                    fill_ctr = 0
                    for qb in range(4):  # 512-wide q block
                        nk = 4 * qb + 4
                        q0 = qb * 512
                        if qb == 1 and pending is not None:
                            norm_flush(*pending)
                            pending = None
                        if last_pair and qb > 0:
                            # attnT rows for t < qb*512 are complete
                            for tt in range(qb * 4 - 4, qb * 4):
                                filler.extend(proj_halves(tt))
                        oT_e = opool_e.tile([65, 512], FP, tag="oe")
                        oT_o = opool_o.tile([65, 512], FP, tag="oo")
                        for tkb in range(0, nk, 2):
                            sps, pts, cs = [], [], []
                            for tk in (tkb, tkb + 1):
                                k0 = tk * 128
                                m = tk - 4 * qb
                                c0 = 0 if m < 1 else 128 * m  # causal trim
                                cs.append((tk, m, c0))
                                sp = spool.tile([128, 1024], FP, tag="s")
                                sps.append(sp)
                                nc.tensor.matmul(
                                    sp[:, c0:512],
                                    kT_sb[0:64, j, k0 : k0 + 128],
                                    qT_sb[0:64, j, q0 + c0 : q0 + 512],
                                    start=True,
                                    stop=True,
                                    tile_position=(0, 0),
                                )
                                last_s = nc.tensor.matmul(
                                    sp[:, 512 + c0 : 1024],
                                    kT_sb[64:128, j, k0 : k0 + 128],
                                    qT_sb[64:128, j, q0 + c0 : q0 + 512],
                                    start=True,
                                    stop=True,
                                    tile_position=(64, 0),
                                )
                            for sp, (tk, m, c0) in zip(sps, cs):
                                pt = ptpool.tile([128, 1024], BF, tag="pt")
                                pts.append(pt)
                                if c0 == 0:
                                    nc.scalar.activation(pt[:], sp[:], AF.Exp, scale=0.125)
                                else:
                                    sp3 = sp[:].rearrange("p (h x) -> p h x", h=2)[:, :, c0:512]
                                    pt3 = pt[:].rearrange("p (h x) -> p h x", h=2)[:, :, c0:512]
                                    nc.scalar.activation(pt3, sp3, AF.Exp, scale=0.125)
                                if m >= 0:  # mask the 128-wide diagonal strip
                                    for hh in (0, 1):
                                        o0 = hh * 512 + c0
                                        nc.vector.tensor_mul(
                                            pt[:, o0 : o0 + 128],
                                            pt[:, o0 : o0 + 128],
                                            maskW_sb[:, 384:512],
                                        )
                            first_av = None
                            for pt, (tk, m, c0) in zip(pts, cs):
                                av = nc.tensor.matmul(
                                    oT_e[:, c0:512],
                                    v_sb[:, tk, 2 * j, :],
                                    pt[:, c0:512],
                                    start=(tk == 0),
                                    stop=(tk == nk - 1),
                                )
                                if first_av is None:
                                    first_av = av
                                    add_dep_helper(
                                        first_av.ins,
                                        last_s.ins,
                                        sync=False,
                                        reason="batch PE mode runs",
                                    )
                                nc.tensor.matmul(
                                    oT_o[:, c0:512],
                                    v_sb[:, tk, 2 * j + 1, :],
                                    pt[:, 512 + c0 : 1024],
                                    start=(tk == 0),
                                    stop=(tk == nk - 1),
                                )
                            fill_ctr += 1
                            if filler and fill_ctr >= fill_iv:
                                fill_ctr = 0
                                filler.pop(0)()
                        if last_pair:
                            norm_last_final(j, qb, ((0, oT_e), (1, oT_o)))
                        else:
                            for hh, oT in ((0, oT_e), (1, oT_o)):
                                ri = qb * 2 + hh
                                stg = stgpool.tile([65, 512], FP, tag="stg")
                                nc.vector.tensor_copy(stg[:], oT[:])
                                nc.sync.dma_start(
                                    out=collect[ri : ri + 1, :], in_=stg[64:65, :]
                                )
                                stgs.append((hh, qb, stg, ri))
                        if j == 0 and qb < 3:
                            # v tiles needed by the next q-block's AVs
                            for tt in range(qb * 4 + 4, qb * 4 + 8):
                                v_unit(tt)
                    # drain any remaining filler units for this pair
                    for h in filler:
                        h()
                    if not last_pair:
                        pending = (j, stgs, collect)
                if pending is not None:
                    norm_flush(*pending)
                    pending = None

            # ---------------- epilogue: remaining proj tiles ------------
            for tt in range(12, 16):
                proj_unit(tt)
    _split_multi_waits(nc)
    return nc


_NC_CACHE = None


def _get_nc():
    global _NC_CACHE
    if _NC_CACHE is None:
        _NC_CACHE = _build_nc()
    return _NC_CACHE


def _mask_w():
    i = np.arange(128)[:, None]
    c = np.arange(896)[None, :]
    return (c >= i + 384).astype(ml_dtypes.bfloat16)


def _in_maps(x, w_qkv, w_proj):
    maskW = _mask_w()
    bf = ml_dtypes.bfloat16
    maps = []
    for core in range(N_CORES):
        b, g = core % 4, core // 4
        maps.append(
            {
                "xT": np.ascontiguousarray(x[b].T).astype(bf),
                "wqk": np.ascontiguousarray(
                    np.concatenate(
                        [
                            w_qkv[:, g * G : (g + 1) * G],
                            w_qkv[:, C + g * G : C + (g + 1) * G],
                        ],
                        axis=1,
                    )
                ).astype(bf),
                "wv": np.ascontiguousarray(
                    w_qkv[:, 2 * C + g * G : 2 * C + (g + 1) * G]
                ).astype(bf),
                "wproj": np.ascontiguousarray(w_proj[g * G : (g + 1) * G, :]).astype(bf),
                "maskW": maskW,
            }
        )
    return maps


def _run(x, w_qkv, w_proj, **spmd_kwargs):
    nc = _get_nc()
    res = run_bass_kernel_spmd(
        nc, _in_maps(x, w_qkv, w_proj), core_ids=list(range(N_CORES)), **spmd_kwargs
    )
    outs = res.results
    full = np.empty((4, T, C), np.float32)
    for b in range(4):
        full[b] = outs[b]["out"] + outs[4 + b]["out"]
    return full, res


def kernel(x, w_qkv, w_proj):
    full, _ = _run(
        np.asarray(x, np.float32),
        np.asarray(w_qkv, np.float32),
        np.asarray(w_proj, np.float32),
    )
    return full
